# revision 32
# baseline (speedup 1.0000x reference)
"""Trainium2 Bass kernel for nn_Critic (branch MLPs -> 255-step LSTM -> head).

Strategy (hardcoded, 8 cores, data-parallel over batch B=512 -> 64/core):
  - Feature-major on chip: vectors are [feature_chunk(128), batch(64)].
  - Gate columns of Wk/Wrk/bl are host-permuted to [f, i, 2*g, o] so that
    tanh(g) = 2*sigmoid(2g) - 1 folds into the one sigmoid pass; the
    i*tanh(g) product is then one fused DVE op ((in0*2-1)*in1).
  - PSUM zz[128, parity, subchunk, step, batch]: subchunk order
    [f0,f1,i0,2g0,i1,2g1,o0,o1]; each parity owns 4 physical banks so the
    bank-granular has_written clear of start=True never touches the other
    group's live cells.  zx = Wk^T x is computed 4 steps at a time with
    N=256 matmuls, spread across the previous group's steps.
  - Per-step chain: f-mms -> sigma_f (feeds off-chain c*=sigma_f) ->
    ig-mms -> sigma_ig -> tm1=(2*sig_g-1)*sig_i (fused) -> c+=tm1 ->
    tanh(c) -> h = sigma_o*tanh(c).  sigma_o runs in ACT slack.
  - Keep-warm dummy matmuls gated on tail events stop the PE from
    downclocking during the per-step wait for h.
"""

import os
os.environ.setdefault("TILE_EXHAUSTIVE_MEMORY_SHARE_CHECK", "1")

import numpy as np
import ml_dtypes

import concourse.bass as bass
import concourse.mybir as mybir
import concourse.tile as tile
from concourse import bacc
from concourse.bass_utils import run_bass_kernel_spmd

BF16 = mybir.dt.bfloat16
F32 = mybir.dt.float32
AF = mybir.ActivationFunctionType

NC = 8          # cores
B = 512
BC = B // NC    # 64 batch per core
T = 255         # real steps
TP = 256        # padded steps
NG = TP // 4    # 64 groups of 4 steps
U = 256
DIN = 256

# tuning flags
KW_DUMMIES = os.environ.get("K_KW", "1") == "1"   # keep-warm dummy matmuls
USE_AFM = os.environ.get("K_AFM", "1") == "1"     # fused (2x-1)*y DVE op
WIDE_ZX = os.environ.get("K_WZX", "1") == "1"     # N=256 zx matmuls
SPLIT_SIG = os.environ.get("K_SSIG", "1") == "1"  # sigma_ig + sigma_o split


def build_nc(use_bias_mm=True):
    nc = bacc.Bacc(None, target_bir_lowering=False)

    d_mot = nc.dram_tensor("mot", [64, BC], BF16, kind="ExternalInput")
    d_rob = nc.dram_tensor("rob", [128, BC], BF16, kind="ExternalInput")
    d_re = nc.dram_tensor("re_", [128, BC], BF16, kind="ExternalInput")
    d_im = nc.dram_tensor("im_", [128, BC], BF16, kind="ExternalInput")
    d_seq = nc.dram_tensor("seq", [2, 128, TP * BC], BF16, kind="ExternalInput")
    d_wm = nc.dram_tensor("wm", [64, 256], BF16, kind="ExternalInput")
    d_wr = nc.dram_tensor("wr", [128, 256], BF16, kind="ExternalInput")
    d_wre = nc.dram_tensor("wre", [128, 128], BF16, kind="ExternalInput")
    d_wim = nc.dram_tensor("wim", [128, 128], BF16, kind="ExternalInput")
    d_wc = nc.dram_tensor("wc", [128, 6, 256], BF16, kind="ExternalInput")
    d_wk = nc.dram_tensor("wk", [128, 2, 1024], BF16, kind="ExternalInput")
    d_wrk = nc.dram_tensor("wrk", [128, 2, 1024], BF16, kind="ExternalInput")
    d_wo = nc.dram_tensor("wo", [128, 2, 1], BF16, kind="ExternalInput")
    d_bm = nc.dram_tensor("bm2", [128, 2], F32, kind="ExternalInput")
    d_br = nc.dram_tensor("br2", [128, 2], F32, kind="ExternalInput")
    d_bre = nc.dram_tensor("bre1", [128, 1], F32, kind="ExternalInput")
    d_bim = nc.dram_tensor("bim1", [128, 1], F32, kind="ExternalInput")
    d_bc = nc.dram_tensor("bc2", [128, 2], F32, kind="ExternalInput")
    d_bo = nc.dram_tensor("bo1", [1, 1], F32, kind="ExternalInput")
    d_blw = nc.dram_tensor("blw8", [8, 128], BF16, kind="ExternalInput")
    d_ind = nc.dram_tensor("ind8", [8, 8 * 256], BF16, kind="ExternalInput")
    d_y = nc.dram_tensor("y", [1, BC], F32, kind="ExternalOutput")
    dbg_t = int(os.environ.get("K_DBG_T", "-1"))
    if dbg_t >= 0:
        d_dh = nc.dram_tensor("dh", [128, 2 * BC], BF16, kind="ExternalOutput")
        d_dc = nc.dram_tensor("dc", [128, 2 * BC], F32, kind="ExternalOutput")
        d_dz = nc.dram_tensor("dz", [128, 8, BC], F32, kind="ExternalOutput")
        d_dg = nc.dram_tensor("dg", [128, 8, BC], F32, kind="ExternalOutput")

    with tile.TileContext(nc) as tc:
        with (
            tc.tile_pool(name="sb", bufs=1) as sb,
            tc.tile_pool(name="rot", bufs=3) as rot,
        ):
            t_wk = sb.tile([128, 2, 1024], BF16, tag="wk")
            t_wrk = sb.tile([128, 2, 1024], BF16, tag="wrk")
            t_blw = sb.tile([8, 128], BF16, tag="blw")
            t_ind = sb.tile([8, 8 * 256], BF16, tag="ind")
            t_seq0 = sb.tile([128, TP * BC], BF16, tag="seq0")
            t_seq1 = sb.tile([128, TP * BC], BF16, tag="seq1")
            t_wm = sb.tile([64, 256], BF16, tag="wm")
            t_wr = sb.tile([128, 256], BF16, tag="wr")
            t_wre = sb.tile([128, 128], BF16, tag="wre")
            t_wim = sb.tile([128, 128], BF16, tag="wim")
            t_wc = sb.tile([128, 6, 256], BF16, tag="wc")
            t_wo = sb.tile([128, 2, 1], BF16, tag="wo")
            t_mot = sb.tile([64, BC], BF16, tag="mot")
            t_rob = sb.tile([128, BC], BF16, tag="rob")
            t_re = sb.tile([128, BC], BF16, tag="re")
            t_im = sb.tile([128, BC], BF16, tag="im")
            t_bm = sb.tile([128, 2], F32, tag="bm")
            t_br = sb.tile([128, 2], F32, tag="br")
            t_bre = sb.tile([128, 1], F32, tag="bre")
            t_bim = sb.tile([128, 1], F32, tag="bim")
            t_bc = sb.tile([128, 2], F32, tag="bc")
            t_bo = sb.tile([1, 1], F32, tag="bo")
            t_h = sb.tile([128, 2 * BC], BF16, tag="h")   # h^T, chunk k at cols k*BC
            t_c = sb.tile([128, 2 * BC], F32, tag="c")    # c^T
            t_cat = sb.tile([128, 6, BC], BF16, tag="cat")
            t_y = sb.tile([1, BC], F32, tag="y")

            nc.sync.dma_start(t_seq0[:, 0:8 * BC], d_seq[0, :, 0:8 * BC])
            nc.sync.dma_start(t_seq1[:, 0:8 * BC], d_seq[1, :, 0:8 * BC])
            nc.sync.dma_start(t_wm[:], d_wm[:])
            nc.sync.dma_start(t_wr[:], d_wr[:])
            nc.sync.dma_start(t_wre[:], d_wre[:])
            nc.sync.dma_start(t_wim[:], d_wim[:])
            nc.sync.dma_start(t_wc[:], d_wc[:])
            nc.sync.dma_start(t_mot[:], d_mot[:])
            nc.sync.dma_start(t_rob[:], d_rob[:])
            nc.sync.dma_start(t_re[:], d_re[:])
            nc.sync.dma_start(t_im[:], d_im[:])
            nc.sync.dma_start(t_bm[:], d_bm[:])
            nc.sync.dma_start(t_br[:], d_br[:])
            nc.sync.dma_start(t_bre[:], d_bre[:])
            nc.sync.dma_start(t_bim[:], d_bim[:])
            nc.sync.dma_start(t_bc[:], d_bc[:])
            nc.sync.dma_start(t_bo[:], d_bo[:])
            nc.sync.dma_start(t_wk[:], d_wk[:])
            nc.sync.dma_start(t_wrk[:], d_wrk[:])
            nc.sync.dma_start(t_blw[:], d_blw[:])
            nc.sync.dma_start(t_ind[:], d_ind[:])
            nc.sync.dma_start(t_wo[:], d_wo[:])
            CH = 16 * BC
            nc.sync.dma_start(t_seq0[:, 8 * BC:CH], d_seq[0, :, 8 * BC:CH])
            nc.sync.dma_start(t_seq1[:, 8 * BC:CH], d_seq[1, :, 8 * BC:CH])
            for ch in range(1, TP // 16):
                nc.sync.dma_start(
                    t_seq0[:, ch * CH:(ch + 1) * CH], d_seq[0, :, ch * CH:(ch + 1) * CH])
                nc.sync.dma_start(
                    t_seq1[:, ch * CH:(ch + 1) * CH], d_seq[1, :, ch * CH:(ch + 1) * CH])
            t_seq = [t_seq0, t_seq1]

            # ---- LSTM recurrence ----
            with tc.tile_pool(name="zp", bufs=1, space="PSUM") as zp:
                # zz[part, bank(gate chunk: f0,f1,i0,i1,g0,g1,o0,o1),
                #    group parity, step in group, batch]
                zz = zp.tile([128, 2, 8, 4, BC], F32, tag="zz")

                # front-end branch MLPs use zz parity-1 step-0 cells as PSUM
                # scratch (group 1's region, re-cleared by its zx pack); the
                # group-0 zx prime then overlaps the ACT relu chain.
                for m in range(2):
                    nc.tensor.matmul(zz[:, 1, m, 0, :], t_wm[:, m * 128:(m + 1) * 128],
                                     t_mot[:], start=(m == 0), stop=True,
                                     skip_group_check=True)
                for m in range(2):
                    nc.tensor.matmul(zz[:, 1, 2 + m, 0, :], t_wr[:, m * 128:(m + 1) * 128],
                                     t_rob[:], start=(m == 0), stop=True,
                                     skip_group_check=True)
                nc.tensor.matmul(zz[:, 1, 4, 0, :], t_wre[:], t_re[:], start=True,
                                 stop=True, skip_group_check=True)
                nc.tensor.matmul(zz[:, 1, 5, 0, :], t_wim[:], t_im[:], start=False,
                                 stop=True, skip_group_check=True)
                for m in range(2):
                    nc.scalar.activation(t_cat[:, m, :], zz[:, 1, m, 0, :], AF.Relu,
                                         bias=t_bm[:, m:m + 1])
                for m in range(2):
                    nc.scalar.activation(t_cat[:, 2 + m, :], zz[:, 1, 2 + m, 0, :],
                                         AF.Relu, bias=t_br[:, m:m + 1])
                nc.scalar.activation(t_cat[:, 4, :], zz[:, 1, 4, 0, :], AF.Relu,
                                     bias=t_bre[:, 0:1])
                nc.scalar.activation(t_cat[:, 5, :], zz[:, 1, 5, 0, :], AF.Relu,
                                     bias=t_bim[:, 0:1])

                def emit_zx_pack(gn, banks):
                    """zx (and bias) for group gn, the given bank pairs;
                    N=256 matmuls covering the group's 4 steps at once.
                    Exactly one start=True per physical bank (first write of
                    the even subchunk) -- start clears the whole bank's
                    has_written state, and each parity owns its own banks."""
                    qn = gn % 2
                    for b in banks:
                        if use_bias_mm:
                            for s in (2 * b, 2 * b + 1):
                                nc.tensor.matmul(
                                    zz[:, qn, s], t_blw[:],
                                    t_ind[:, s * 256:(s + 1) * 256],
                                    start=(s == 2 * b), stop=False,
                                    skip_group_check=True)
                        for s in (2 * b, 2 * b + 1):
                            for k in range(2):
                                if WIDE_ZX:
                                    nc.tensor.matmul(
                                        zz[:, qn, s],
                                        t_wk[:, k, s * 128:(s + 1) * 128],
                                        t_seq[k][:, gn * 4 * BC:(gn + 1) * 4 * BC],
                                        start=(not use_bias_mm and k == 0
                                               and s == 2 * b),
                                        stop=False, skip_group_check=True)
                                else:
                                    for p in range(4):
                                        nc.tensor.matmul(
                                            zz[:, qn, s, p, :],
                                            t_wk[:, k, s * 128:(s + 1) * 128],
                                            t_seq[k][:, (gn * 4 + p) * BC:
                                                      (gn * 4 + p + 1) * BC],
                                            start=(not use_bias_mm and k == 0
                                                   and s == 2 * b and p == 0),
                                            stop=False, skip_group_check=True)

                def emit_step(t):
                    g, q, p = t // 4, (t // 4) % 2, t % 4
                    gb = rot.tile([128, 8, BC], F32, tag="gb")
                    gbo = rot.tile([128, 2, BC], BF16, tag="gbo")
                    tm1 = rot.tile([128, 2, BC], F32, tag="tm1")
                    tm2 = rot.tile([128, 2 * BC], F32, tag="tm2")
                    tmc = rot.tile([128, 2 * BC], BF16, tag="tmc")
                    t_acc = rot.tile([128, 1], F32, tag="acc")
                    if t == dbg_t:
                        dbg_gb.append(gb)
                    # recurrent matmuls: f subchunks first, then i,g, then o
                    for s in range(8):
                        for k in range(2):
                            nc.tensor.matmul(
                                zz[:, q, s, p, :],
                                t_wrk[:, k, s * 128:(s + 1) * 128],
                                t_h[:, k * BC:(k + 1) * BC],
                                start=False, stop=(k == 1),
                                skip_group_check=True)
                    # ACT queue: sigma_f, sigma_igo, tanh(c)
                    nc.scalar.activation(gb[:, 0:2, :], zz[:, q, 0:2, p, :],
                                         AF.Sigmoid)
                    if SPLIT_SIG:
                        nc.scalar.activation(gb[:, 2:6, :], zz[:, q, 2:6, p, :],
                                             AF.Sigmoid)
                        nc.scalar.activation(gbo[:, :, :], zz[:, q, 6:8, p, :],
                                             AF.Sigmoid)
                    else:
                        nc.scalar.activation(gb[:, 2:6, :], zz[:, q, 2:6, p, :],
                                             AF.Sigmoid)
                        nc.scalar.activation(gbo[:, :, :], zz[:, q, 6:8, p, :],
                                             AF.Sigmoid)
                    # DVE: tm2 = sig_f*c ; tm1 = (2*sig_2g-1)*sig_i ;
                    #      c = tm2 + tm1 ; h = sig_o * tanh(c)
                    nc.vector.tensor_mul(tm2[:, :], gb[:, 0:2, :], t_c[:, :])
                    if USE_AFM:
                        nc.vector.affine_mul_reduce(
                            tm1[:, :, :], t_acc[:, 0:1], gb[:, 3:7:2, :],
                            gb[:, 2:6:2, :], 2.0, -1.0)
                    else:
                        nc.vector.tensor_scalar(
                            tm1[:, :, :], gb[:, 3:7:2, :], 2.0, -1.0,
                            mybir.AluOpType.mult, mybir.AluOpType.add)
                        nc.vector.tensor_mul(tm1[:, :, :], tm1[:, :, :],
                                             gb[:, 2:6:2, :])
                    nc.vector.tensor_add(t_c[:, :], tm2[:, :], tm1[:, :, :])
                    nc.scalar.activation(tmc[:, :], t_c[:, :], AF.Tanh)
                    nc.vector.tensor_mul(t_h[:, :], gbo[:, :, :], tmc[:, :])
                    # zx for the next group, one bank pair per step
                    if g + 1 < NG:
                        emit_zx_pack(g + 1, (p,))
                    # keep-warm dummies: tiny matmuls gated on tail results so
                    # the PE never idles long enough to downclock.  They
                    # accumulate (start=False: no bank clear!) onto dead PSUM
                    # cells of this step's already-consumed z.
                    if KW_DUMMIES:
                        # event-gated keep-warm ladder: fat fp32 fillers early
                        # in the tail, short bf16 ones near the h handoff
                        # [event-gated sync, static filler] pairs: the gated
                        # mm aligns to a tail event; static mms behind it in
                        # the in-order queue extend the busy stretch.
                        sf32 = t_seq0[:, 0:128].bitcast(F32)
                        def kw(src_, st_, s, fills):
                            nc.tensor.matmul(
                                zz[0:1, q, s, p, 0:BC], st_, src_,
                                start=False, stop=True, skip_group_check=True)
                            for _ in range(fills):
                                nc.tensor.matmul(
                                    zz[0:1, q, s, p, 0:BC], t_bm[:, 0:1],
                                    sf32[:, 0:BC],
                                    start=False, stop=True,
                                    skip_group_check=True)
                        kw(gb[:, 2, 0:BC], t_bm[:, 0:1], 2, 1)
                        kw(tm1[:, 0, 0:BC], t_bm[:, 0:1], 3, 3)
                        kw(t_c[:, 0:BC], t_bm[:, 0:1], 4, 2)
                        kw(tmc[:, 0:BC], t_wk[:, 0, 0:1], 5, 3)

                dbg_gb = []
                emit_zx_pack(0, range(4))
                # state = relu(cat @ Wc + bc) -> h0, c0 (PE queue: after the
                # zx prime, so the prime overlaps the relu chain above)
                for mo in range(2):
                    for kc in range(6):
                        nc.tensor.matmul(
                            zz[:, 1, 6 + mo, 0, :],
                            t_wc[:, kc, mo * 128:(mo + 1) * 128],
                            t_cat[:, kc, :],
                            start=(mo == 0 and kc == 0), stop=(kc == 5),
                            skip_group_check=True)
                for mo in range(2):
                    nc.scalar.activation(t_h[:, mo * BC:(mo + 1) * BC],
                                         zz[:, 1, 6 + mo, 0, :],
                                         AF.Relu, bias=t_bc[:, mo:mo + 1])
                    nc.scalar.activation(t_c[:, mo * BC:(mo + 1) * BC],
                                         zz[:, 1, 6 + mo, 0, :],
                                         AF.Relu, bias=t_bc[:, mo:mo + 1])
                for t in range(T):
                    emit_step(t)
                    if t == dbg_t:
                        nc.sync.dma_start(d_dh[:], t_h[:])
                        nc.sync.dma_start(d_dc[:], t_c[:])
                        dq, dp = (t // 4) % 2, t % 4
                        dzt = rot.tile([128, 8, BC], F32, tag="dzt")
                        for j in range(8):
                            nc.vector.tensor_copy(dzt[:, j, :], zz[:, dq, j, dp, :])
                        nc.sync.dma_start(d_dz[:], dzt[:])
                        nc.sync.dma_start(d_dg[:], dbg_gb[0][:])

            # ---- output head ----
            with tc.tile_pool(name="hp", bufs=1, space="PSUM") as hp:
                py = hp.tile([1, BC], F32, tag="py")
                for k in range(2):
                    nc.tensor.matmul(py[:], t_wo[:, k, :],
                                     t_h[:, k * BC:(k + 1) * BC],
                                     start=(k == 0), stop=(k == 1))
                nc.scalar.activation(t_y[:], py[:], AF.Relu, bias=t_bo[:, 0:1])
            nc.sync.dma_start(d_y[:], t_y[:])

    nc.compile()
    return nc


_NC_CACHE = None


def _prep_inputs(inputs):
    """Shard + lay out the full-problem inputs into 8 per-core in_maps."""
    bf = ml_dtypes.bfloat16
    f32 = np.float32

    hist = np.asarray(inputs["history"], f32)     # [B, 128, 256]
    act = np.asarray(inputs["action"], f32)       # [B, 128, 256]
    seq = np.concatenate([hist[:, :127], act], axis=1)          # [B, 255, 256]
    seq = np.concatenate(
        [seq, np.zeros((B, 1, DIN), f32)], axis=1)              # [B, 256, 256]

    def gate_perm(w):
        """Permute gate cols [i,f,g,o] -> [f0,f1,i0,2g0,i1,2g1,o0,o1]
        (last axis, 8x128 subchunks)."""
        return np.concatenate(
            [w[..., 256:512], w[..., 0:128], 2.0 * w[..., 512:640],
             w[..., 128:256], 2.0 * w[..., 640:768], w[..., 768:1024]],
            axis=-1)

    Wk = gate_perm(np.asarray(inputs["Wk"], f32))    # [256, 1024]
    Wrk = gate_perm(np.asarray(inputs["Wrk"], f32))
    bl = gate_perm(np.asarray(inputs["bl"], f32).reshape(1, 1024))[0]
    wk_p = np.ascontiguousarray(
        Wk.reshape(2, 128, 1024).transpose(1, 0, 2)).astype(bf)   # [128,2,1024]
    wrk_p = np.ascontiguousarray(
        Wrk.reshape(2, 128, 1024).transpose(1, 0, 2)).astype(bf)
    blw8 = np.ascontiguousarray(bl.reshape(8, 128)).astype(bf)    # [8,128]
    ind8 = np.zeros((8, 8 * 256), f32)
    for j in range(8):
        ind8[j, j * 256:(j + 1) * 256] = 1.0
    ind8 = ind8.astype(bf)
    Wc = np.asarray(inputs["Wc"], f32)            # [768, 256]
    wc_p = np.ascontiguousarray(
        Wc.reshape(6, 128, 256).transpose(1, 0, 2)).astype(bf)    # [128,6,256]
    Wo = np.asarray(inputs["Wo"], f32)            # [256, 1]
    wo_p = np.ascontiguousarray(
        Wo.reshape(2, 128, 1).transpose(1, 0, 2)).astype(bf)      # [128,2,1]

    def bias2(v, chunks):
        return np.ascontiguousarray(np.asarray(v, f32).reshape(chunks, 128).T)

    shared = {
        "wm": np.asarray(inputs["Wm"], f32).astype(bf),
        "wr": np.asarray(inputs["Wr"], f32).astype(bf),
        "wre": np.asarray(inputs["Wre"], f32).astype(bf),
        "wim": np.asarray(inputs["Wim"], f32).astype(bf),
        "wc": wc_p, "wk": wk_p, "wrk": wrk_p, "wo": wo_p,
        "bm2": bias2(inputs["bm"], 2), "br2": bias2(inputs["br"], 2),
        "bre1": bias2(inputs["bre"], 1), "bim1": bias2(inputs["bim"], 1),
        "bc2": bias2(inputs["bc"], 2),
        "bo1": np.asarray(inputs["bo"], f32).reshape(1, 1),
        "blw8": blw8, "ind8": ind8,
    }

    mot = np.asarray(inputs["motion_state"], f32)
    rob = np.asarray(inputs["robot_state"], f32)
    real = np.concatenate([np.asarray(inputs["osc_state_real"], f32),
                           np.asarray(inputs["osc_real"], f32)], -1)
    imag = np.concatenate([np.asarray(inputs["osc_state_imag"], f32),
                           np.asarray(inputs["osc_imag"], f32)], -1)

    in_maps = []
    for c in range(NC):
        sl = slice(c * BC, (c + 1) * BC)
        # on-chip col = t*64 + b  (plain t-major)
        sc = seq[sl].reshape(BC, TP, 2, 128)           # [b, t, fk, fp]
        sc = np.ascontiguousarray(sc.transpose(2, 3, 1, 0)).astype(bf)
        m = dict(shared)
        m["seq"] = np.ascontiguousarray(sc.reshape(2, 128, TP * BC))
        m["mot"] = np.ascontiguousarray(mot[sl].T).astype(bf)
        m["rob"] = np.ascontiguousarray(rob[sl].T).astype(bf)
        m["re_"] = np.ascontiguousarray(real[sl].T).astype(bf)
        m["im_"] = np.ascontiguousarray(imag[sl].T).astype(bf)
        in_maps.append(m)
    return in_maps


def kernel(**inputs):
    global _NC_CACHE
    use_bias_mm = bool(np.any(np.asarray(inputs["bl"])))
    if _NC_CACHE is None or _NC_CACHE[1] != use_bias_mm:
        _NC_CACHE = (build_nc(use_bias_mm), use_bias_mm)
    in_maps = _prep_inputs(inputs)
    res = run_bass_kernel_spmd(_NC_CACHE[0], in_maps, core_ids=list(range(NC)))
    out = np.concatenate(
        [np.asarray(res.results[c]["y"], np.float32).T for c in range(NC)], axis=0)
    return out  # [512, 1] float32


# revision 33
# speedup vs baseline: 1.0372x; 1.0372x over previous
"""Trainium2 Bass kernel for nn_Critic (branch MLPs -> 255-step LSTM -> head).

Strategy (hardcoded, 8 cores, data-parallel over batch B=512 -> 64/core):
  - Feature-major on chip: vectors are [feature_chunk(128), batch(64)].
  - Gate columns of Wk/Wrk/bl are host-permuted to [f, i, 2*g, o] so that
    tanh(g) = 2*sigmoid(2g) - 1 folds into the one sigmoid pass; the
    i*tanh(g) product is then one fused DVE op ((in0*2-1)*in1).
  - PSUM zz[128, parity, subchunk, step, batch]: subchunk order
    [f0,f1,i0,2g0,i1,2g1,o0,o1]; each parity owns 4 physical banks so the
    bank-granular has_written clear of start=True never touches the other
    group's live cells.  zx = Wk^T x is computed 4 steps at a time with
    N=256 matmuls, spread across the previous group's steps.
  - Per-step chain: f-mms -> sigma_f (feeds off-chain c*=sigma_f) ->
    ig-mms -> sigma_ig -> tm1=(2*sig_g-1)*sig_i (fused) -> c+=tm1 ->
    tanh(c) -> h = sigma_o*tanh(c).  sigma_o runs in ACT slack.
  - Keep-warm dummy matmuls gated on tail events stop the PE from
    downclocking during the per-step wait for h.
"""

import os
os.environ.setdefault("TILE_EXHAUSTIVE_MEMORY_SHARE_CHECK", "1")

import numpy as np
import ml_dtypes

import concourse.bass as bass
import concourse.mybir as mybir
import concourse.tile as tile
from concourse import bacc
from concourse.bass_utils import run_bass_kernel_spmd

BF16 = mybir.dt.bfloat16
F32 = mybir.dt.float32
AF = mybir.ActivationFunctionType

NC = 8          # cores
B = 512
BC = B // NC    # 64 batch per core
T = 255         # real steps
TP = 256        # padded steps
NG = TP // 4    # 64 groups of 4 steps
U = 256
DIN = 256

# tuning flags
KW_DUMMIES = os.environ.get("K_KW", "1") == "1"   # keep-warm dummy matmuls
USE_AFM = os.environ.get("K_AFM", "1") == "1"     # fused (2x-1)*y DVE op
WIDE_ZX = os.environ.get("K_WZX", "1") == "1"     # N=256 zx matmuls
SPLIT_SIG = os.environ.get("K_SSIG", "1") == "1"  # sigma_ig + sigma_o split


def build_nc(use_bias_mm=True):
    nc = bacc.Bacc(None, target_bir_lowering=False)

    d_mot = nc.dram_tensor("mot", [64, BC], BF16, kind="ExternalInput")
    d_rob = nc.dram_tensor("rob", [128, BC], BF16, kind="ExternalInput")
    d_re = nc.dram_tensor("re_", [128, BC], BF16, kind="ExternalInput")
    d_im = nc.dram_tensor("im_", [128, BC], BF16, kind="ExternalInput")
    d_seq = nc.dram_tensor("seq", [2, 128, TP * BC], BF16, kind="ExternalInput")
    d_wm = nc.dram_tensor("wm", [64, 256], BF16, kind="ExternalInput")
    d_wr = nc.dram_tensor("wr", [128, 256], BF16, kind="ExternalInput")
    d_wre = nc.dram_tensor("wre", [128, 128], BF16, kind="ExternalInput")
    d_wim = nc.dram_tensor("wim", [128, 128], BF16, kind="ExternalInput")
    d_wc = nc.dram_tensor("wc", [128, 6, 256], BF16, kind="ExternalInput")
    d_wk = nc.dram_tensor("wk", [128, 2, 1024], BF16, kind="ExternalInput")
    d_wrk = nc.dram_tensor("wrk", [128, 2, 1024], BF16, kind="ExternalInput")
    d_wo = nc.dram_tensor("wo", [128, 2, 1], BF16, kind="ExternalInput")
    d_bm = nc.dram_tensor("bm2", [128, 2], F32, kind="ExternalInput")
    d_br = nc.dram_tensor("br2", [128, 2], F32, kind="ExternalInput")
    d_bre = nc.dram_tensor("bre1", [128, 1], F32, kind="ExternalInput")
    d_bim = nc.dram_tensor("bim1", [128, 1], F32, kind="ExternalInput")
    d_bc = nc.dram_tensor("bc2", [128, 2], F32, kind="ExternalInput")
    d_bo = nc.dram_tensor("bo1", [1, 1], F32, kind="ExternalInput")
    d_blw = nc.dram_tensor("blw8", [8, 128], BF16, kind="ExternalInput")
    d_ind = nc.dram_tensor("ind8", [8, 8 * 256], BF16, kind="ExternalInput")
    d_y = nc.dram_tensor("y", [1, BC], F32, kind="ExternalOutput")
    dbg_t = int(os.environ.get("K_DBG_T", "-1"))
    if dbg_t >= 0:
        d_dh = nc.dram_tensor("dh", [128, 2 * BC], BF16, kind="ExternalOutput")
        d_dc = nc.dram_tensor("dc", [128, 2 * BC], F32, kind="ExternalOutput")
        d_dz = nc.dram_tensor("dz", [128, 8, BC], F32, kind="ExternalOutput")
        d_dg = nc.dram_tensor("dg", [128, 8, BC], F32, kind="ExternalOutput")

    with tile.TileContext(nc) as tc:
        with (
            tc.tile_pool(name="sb", bufs=1) as sb,
            tc.tile_pool(name="rot", bufs=3) as rot,
        ):
            t_wk = sb.tile([128, 2, 1024], BF16, tag="wk")
            t_wrk = sb.tile([128, 2, 1024], BF16, tag="wrk")
            t_blw = sb.tile([8, 128], BF16, tag="blw")
            t_ind = sb.tile([8, 8 * 256], BF16, tag="ind")
            t_seq0 = sb.tile([128, TP * BC], BF16, tag="seq0")
            t_seq1 = sb.tile([128, TP * BC], BF16, tag="seq1")
            t_wm = sb.tile([64, 256], BF16, tag="wm")
            t_wr = sb.tile([128, 256], BF16, tag="wr")
            t_wre = sb.tile([128, 128], BF16, tag="wre")
            t_wim = sb.tile([128, 128], BF16, tag="wim")
            t_wc = sb.tile([128, 6, 256], BF16, tag="wc")
            t_wo = sb.tile([128, 2, 1], BF16, tag="wo")
            t_mot = sb.tile([64, BC], BF16, tag="mot")
            t_rob = sb.tile([128, BC], BF16, tag="rob")
            t_re = sb.tile([128, BC], BF16, tag="re")
            t_im = sb.tile([128, BC], BF16, tag="im")
            t_bm = sb.tile([128, 2], F32, tag="bm")
            t_br = sb.tile([128, 2], F32, tag="br")
            t_bre = sb.tile([128, 1], F32, tag="bre")
            t_bim = sb.tile([128, 1], F32, tag="bim")
            t_bc = sb.tile([128, 2], F32, tag="bc")
            t_bo = sb.tile([1, 1], F32, tag="bo")
            t_h = sb.tile([128, 2 * BC], BF16, tag="h")   # h^T, chunk k at cols k*BC
            t_c = sb.tile([128, 2 * BC], F32, tag="c")    # c^T
            t_cat = sb.tile([128, 6, BC], BF16, tag="cat")
            t_y = sb.tile([1, BC], F32, tag="y")

            nc.sync.dma_start(t_seq0[:, 0:8 * BC], d_seq[0, :, 0:8 * BC])
            nc.sync.dma_start(t_seq1[:, 0:8 * BC], d_seq[1, :, 0:8 * BC])
            nc.sync.dma_start(t_wm[:], d_wm[:])
            nc.sync.dma_start(t_wr[:], d_wr[:])
            nc.sync.dma_start(t_wre[:], d_wre[:])
            nc.sync.dma_start(t_wim[:], d_wim[:])
            nc.sync.dma_start(t_wc[:], d_wc[:])
            nc.sync.dma_start(t_mot[:], d_mot[:])
            nc.sync.dma_start(t_rob[:], d_rob[:])
            nc.sync.dma_start(t_re[:], d_re[:])
            nc.sync.dma_start(t_im[:], d_im[:])
            nc.sync.dma_start(t_bm[:], d_bm[:])
            nc.sync.dma_start(t_br[:], d_br[:])
            nc.sync.dma_start(t_bre[:], d_bre[:])
            nc.sync.dma_start(t_bim[:], d_bim[:])
            nc.sync.dma_start(t_bc[:], d_bc[:])
            nc.sync.dma_start(t_bo[:], d_bo[:])
            nc.sync.dma_start(t_wk[:], d_wk[:])
            nc.sync.dma_start(t_wrk[:], d_wrk[:])
            nc.sync.dma_start(t_blw[:], d_blw[:])
            nc.sync.dma_start(t_ind[:], d_ind[:])
            nc.sync.dma_start(t_wo[:], d_wo[:])
            CH = 16 * BC
            nc.sync.dma_start(t_seq0[:, 8 * BC:CH], d_seq[0, :, 8 * BC:CH])
            nc.sync.dma_start(t_seq1[:, 8 * BC:CH], d_seq[1, :, 8 * BC:CH])
            for ch in range(1, TP // 16):
                nc.sync.dma_start(
                    t_seq0[:, ch * CH:(ch + 1) * CH], d_seq[0, :, ch * CH:(ch + 1) * CH])
                nc.sync.dma_start(
                    t_seq1[:, ch * CH:(ch + 1) * CH], d_seq[1, :, ch * CH:(ch + 1) * CH])
            t_seq = [t_seq0, t_seq1]

            # ---- LSTM recurrence ----
            with tc.tile_pool(name="zp", bufs=1, space="PSUM") as zp:
                # zz[part, bank(gate chunk: f0,f1,i0,i1,g0,g1,o0,o1),
                #    group parity, step in group, batch]
                zz = zp.tile([128, 2, 8, 4, BC], F32, tag="zz")

                # front-end branch MLPs use zz parity-1 step-0 cells as PSUM
                # scratch (group 1's region, re-cleared by its zx pack); the
                # group-0 zx prime then overlaps the ACT relu chain.
                for m in range(2):
                    nc.tensor.matmul(zz[:, 1, m, 0, :], t_wm[:, m * 128:(m + 1) * 128],
                                     t_mot[:], start=(m == 0), stop=True,
                                     skip_group_check=True)
                for m in range(2):
                    nc.tensor.matmul(zz[:, 1, 2 + m, 0, :], t_wr[:, m * 128:(m + 1) * 128],
                                     t_rob[:], start=(m == 0), stop=True,
                                     skip_group_check=True)
                nc.tensor.matmul(zz[:, 1, 4, 0, :], t_wre[:], t_re[:], start=True,
                                 stop=True, skip_group_check=True)
                nc.tensor.matmul(zz[:, 1, 5, 0, :], t_wim[:], t_im[:], start=False,
                                 stop=True, skip_group_check=True)
                for m in range(2):
                    nc.scalar.activation(t_cat[:, m, :], zz[:, 1, m, 0, :], AF.Relu,
                                         bias=t_bm[:, m:m + 1])
                for m in range(2):
                    nc.scalar.activation(t_cat[:, 2 + m, :], zz[:, 1, 2 + m, 0, :],
                                         AF.Relu, bias=t_br[:, m:m + 1])
                nc.scalar.activation(t_cat[:, 4, :], zz[:, 1, 4, 0, :], AF.Relu,
                                     bias=t_bre[:, 0:1])
                nc.scalar.activation(t_cat[:, 5, :], zz[:, 1, 5, 0, :], AF.Relu,
                                     bias=t_bim[:, 0:1])

                def emit_zx_pack(gn, banks):
                    """zx (and bias) for group gn, the given bank pairs;
                    N=256 matmuls covering the group's 4 steps at once.
                    Exactly one start=True per physical bank (first write of
                    the even subchunk) -- start clears the whole bank's
                    has_written state, and each parity owns its own banks."""
                    qn = gn % 2
                    for b in banks:
                        if use_bias_mm:
                            for s in (2 * b, 2 * b + 1):
                                nc.tensor.matmul(
                                    zz[:, qn, s], t_blw[:],
                                    t_ind[:, s * 256:(s + 1) * 256],
                                    start=(s == 2 * b), stop=False,
                                    skip_group_check=True)
                        for s in (2 * b, 2 * b + 1):
                            for k in range(2):
                                if WIDE_ZX:
                                    nc.tensor.matmul(
                                        zz[:, qn, s],
                                        t_wk[:, k, s * 128:(s + 1) * 128],
                                        t_seq[k][:, gn * 4 * BC:(gn + 1) * 4 * BC],
                                        start=(not use_bias_mm and k == 0
                                               and s == 2 * b),
                                        stop=False, skip_group_check=True)
                                else:
                                    for p in range(4):
                                        nc.tensor.matmul(
                                            zz[:, qn, s, p, :],
                                            t_wk[:, k, s * 128:(s + 1) * 128],
                                            t_seq[k][:, (gn * 4 + p) * BC:
                                                      (gn * 4 + p + 1) * BC],
                                            start=(not use_bias_mm and k == 0
                                                   and s == 2 * b and p == 0),
                                            stop=False, skip_group_check=True)

                def emit_step(t):
                    g, q, p = t // 4, (t // 4) % 2, t % 4
                    gb = rot.tile([128, 8, BC], F32, tag="gb")
                    gbo = rot.tile([128, 2, BC], BF16, tag="gbo")
                    tm1 = rot.tile([128, 2, BC], F32, tag="tm1")
                    tm2 = rot.tile([128, 2 * BC], F32, tag="tm2")
                    tmc = rot.tile([128, 2 * BC], BF16, tag="tmc")
                    t_acc = rot.tile([128, 1], F32, tag="acc")
                    if t == dbg_t:
                        dbg_gb.append(gb)
                    # recurrent matmuls: f subchunks first, then i,g, then o
                    for s in range(8):
                        for k in range(2):
                            nc.tensor.matmul(
                                zz[:, q, s, p, :],
                                t_wrk[:, k, s * 128:(s + 1) * 128],
                                t_h[:, k * BC:(k + 1) * BC],
                                start=False, stop=(k == 1),
                                skip_group_check=True)
                    # ACT queue: sigma_f, sigma_igo, tanh(c)
                    nc.scalar.activation(gb[:, 0:2, :], zz[:, q, 0:2, p, :],
                                         AF.Sigmoid)
                    if SPLIT_SIG:
                        nc.scalar.activation(gb[:, 2:6, :], zz[:, q, 2:6, p, :],
                                             AF.Sigmoid)
                        nc.scalar.activation(gbo[:, :, :], zz[:, q, 6:8, p, :],
                                             AF.Sigmoid)
                    else:
                        nc.scalar.activation(gb[:, 2:6, :], zz[:, q, 2:6, p, :],
                                             AF.Sigmoid)
                        nc.scalar.activation(gbo[:, :, :], zz[:, q, 6:8, p, :],
                                             AF.Sigmoid)
                    # DVE: tm2 = sig_f*c ; tm1 = (2*sig_2g-1)*sig_i ;
                    #      c = tm2 + tm1 ; h = sig_o * tanh(c)
                    nc.vector.tensor_mul(tm2[:, :], gb[:, 0:2, :], t_c[:, :])
                    if USE_AFM:
                        nc.vector.affine_mul_reduce(
                            tm1[:, :, :], t_acc[:, 0:1], gb[:, 3:7:2, :],
                            gb[:, 2:6:2, :], 2.0, -1.0)
                    else:
                        nc.vector.tensor_scalar(
                            tm1[:, :, :], gb[:, 3:7:2, :], 2.0, -1.0,
                            mybir.AluOpType.mult, mybir.AluOpType.add)
                        nc.vector.tensor_mul(tm1[:, :, :], tm1[:, :, :],
                                             gb[:, 2:6:2, :])
                    nc.vector.tensor_add(t_c[:, :], tm2[:, :], tm1[:, :, :])
                    nc.scalar.activation(tmc[:, :], t_c[:, :], AF.Tanh)
                    nc.vector.tensor_mul(t_h[:, :], gbo[:, :, :], tmc[:, :])
                    # zx for the next group, one bank pair per step
                    if g + 1 < NG:
                        emit_zx_pack(g + 1, (p,))
                    # keep-warm dummies: tiny matmuls gated on tail results so
                    # the PE never idles long enough to downclock.  They
                    # accumulate (start=False: no bank clear!) onto dead PSUM
                    # cells of this step's already-consumed z.
                    if KW_DUMMIES:
                        # event-gated keep-warm ladder: fat fp32 fillers early
                        # in the tail, short bf16 ones near the h handoff
                        # [event-gated sync, static filler] pairs: the gated
                        # mm aligns to a tail event; static mms behind it in
                        # the in-order queue extend the busy stretch.
                        sf32 = t_seq0[:, 0:128].bitcast(F32)
                        def kw(src_, st_, s, fills):
                            nc.tensor.matmul(
                                zz[0:1, q, s, p, 0:BC], st_, src_,
                                start=False, stop=True, skip_group_check=True)
                            for _ in range(fills):
                                nc.tensor.matmul(
                                    zz[0:1, q, s, p, 0:BC], t_bm[:, 0:1],
                                    sf32[:, 0:BC],
                                    start=False, stop=True,
                                    skip_group_check=True)
                        kw(gb[:, 2, 0:BC], t_bm[:, 0:1], 2, 1)
                        kw(tm1[:, 0, 0:BC], t_bm[:, 0:1], 3, 3)
                        kw(t_c[:, 0:BC], t_bm[:, 0:1], 4, 2)
                        kw(tmc[:, 0:BC], t_wk[:, 0, 0:1], 5, 2)

                dbg_gb = []
                emit_zx_pack(0, range(4))
                # state = relu(cat @ Wc + bc) -> h0, c0 (PE queue: after the
                # zx prime, so the prime overlaps the relu chain above)
                for mo in range(2):
                    for kc in range(6):
                        nc.tensor.matmul(
                            zz[:, 1, 6 + mo, 0, :],
                            t_wc[:, kc, mo * 128:(mo + 1) * 128],
                            t_cat[:, kc, :],
                            start=(mo == 0 and kc == 0), stop=(kc == 5),
                            skip_group_check=True)
                for mo in range(2):
                    nc.scalar.activation(t_h[:, mo * BC:(mo + 1) * BC],
                                         zz[:, 1, 6 + mo, 0, :],
                                         AF.Relu, bias=t_bc[:, mo:mo + 1])
                    nc.scalar.activation(t_c[:, mo * BC:(mo + 1) * BC],
                                         zz[:, 1, 6 + mo, 0, :],
                                         AF.Relu, bias=t_bc[:, mo:mo + 1])
                for t in range(T):
                    emit_step(t)
                    if t == dbg_t:
                        nc.sync.dma_start(d_dh[:], t_h[:])
                        nc.sync.dma_start(d_dc[:], t_c[:])
                        dq, dp = (t // 4) % 2, t % 4
                        dzt = rot.tile([128, 8, BC], F32, tag="dzt")
                        for j in range(8):
                            nc.vector.tensor_copy(dzt[:, j, :], zz[:, dq, j, dp, :])
                        nc.sync.dma_start(d_dz[:], dzt[:])
                        nc.sync.dma_start(d_dg[:], dbg_gb[0][:])

            # ---- output head ----
            with tc.tile_pool(name="hp", bufs=1, space="PSUM") as hp:
                py = hp.tile([1, BC], F32, tag="py")
                for k in range(2):
                    nc.tensor.matmul(py[:], t_wo[:, k, :],
                                     t_h[:, k * BC:(k + 1) * BC],
                                     start=(k == 0), stop=(k == 1))
                nc.scalar.activation(t_y[:], py[:], AF.Relu, bias=t_bo[:, 0:1])
            nc.sync.dma_start(d_y[:], t_y[:])

    nc.compile()
    return nc


_NC_CACHE = None


def _prep_inputs(inputs):
    """Shard + lay out the full-problem inputs into 8 per-core in_maps."""
    bf = ml_dtypes.bfloat16
    f32 = np.float32

    hist = np.asarray(inputs["history"], f32)     # [B, 128, 256]
    act = np.asarray(inputs["action"], f32)       # [B, 128, 256]
    seq = np.concatenate([hist[:, :127], act], axis=1)          # [B, 255, 256]
    seq = np.concatenate(
        [seq, np.zeros((B, 1, DIN), f32)], axis=1)              # [B, 256, 256]

    def gate_perm(w):
        """Permute gate cols [i,f,g,o] -> [f0,f1,i0,2g0,i1,2g1,o0,o1]
        (last axis, 8x128 subchunks)."""
        return np.concatenate(
            [w[..., 256:512], w[..., 0:128], 2.0 * w[..., 512:640],
             w[..., 128:256], 2.0 * w[..., 640:768], w[..., 768:1024]],
            axis=-1)

    Wk = gate_perm(np.asarray(inputs["Wk"], f32))    # [256, 1024]
    Wrk = gate_perm(np.asarray(inputs["Wrk"], f32))
    bl = gate_perm(np.asarray(inputs["bl"], f32).reshape(1, 1024))[0]
    wk_p = np.ascontiguousarray(
        Wk.reshape(2, 128, 1024).transpose(1, 0, 2)).astype(bf)   # [128,2,1024]
    wrk_p = np.ascontiguousarray(
        Wrk.reshape(2, 128, 1024).transpose(1, 0, 2)).astype(bf)
    blw8 = np.ascontiguousarray(bl.reshape(8, 128)).astype(bf)    # [8,128]
    ind8 = np.zeros((8, 8 * 256), f32)
    for j in range(8):
        ind8[j, j * 256:(j + 1) * 256] = 1.0
    ind8 = ind8.astype(bf)
    Wc = np.asarray(inputs["Wc"], f32)            # [768, 256]
    wc_p = np.ascontiguousarray(
        Wc.reshape(6, 128, 256).transpose(1, 0, 2)).astype(bf)    # [128,6,256]
    Wo = np.asarray(inputs["Wo"], f32)            # [256, 1]
    wo_p = np.ascontiguousarray(
        Wo.reshape(2, 128, 1).transpose(1, 0, 2)).astype(bf)      # [128,2,1]

    def bias2(v, chunks):
        return np.ascontiguousarray(np.asarray(v, f32).reshape(chunks, 128).T)

    shared = {
        "wm": np.asarray(inputs["Wm"], f32).astype(bf),
        "wr": np.asarray(inputs["Wr"], f32).astype(bf),
        "wre": np.asarray(inputs["Wre"], f32).astype(bf),
        "wim": np.asarray(inputs["Wim"], f32).astype(bf),
        "wc": wc_p, "wk": wk_p, "wrk": wrk_p, "wo": wo_p,
        "bm2": bias2(inputs["bm"], 2), "br2": bias2(inputs["br"], 2),
        "bre1": bias2(inputs["bre"], 1), "bim1": bias2(inputs["bim"], 1),
        "bc2": bias2(inputs["bc"], 2),
        "bo1": np.asarray(inputs["bo"], f32).reshape(1, 1),
        "blw8": blw8, "ind8": ind8,
    }

    mot = np.asarray(inputs["motion_state"], f32)
    rob = np.asarray(inputs["robot_state"], f32)
    real = np.concatenate([np.asarray(inputs["osc_state_real"], f32),
                           np.asarray(inputs["osc_real"], f32)], -1)
    imag = np.concatenate([np.asarray(inputs["osc_state_imag"], f32),
                           np.asarray(inputs["osc_imag"], f32)], -1)

    in_maps = []
    for c in range(NC):
        sl = slice(c * BC, (c + 1) * BC)
        # on-chip col = t*64 + b  (plain t-major)
        sc = seq[sl].reshape(BC, TP, 2, 128)           # [b, t, fk, fp]
        sc = np.ascontiguousarray(sc.transpose(2, 3, 1, 0)).astype(bf)
        m = dict(shared)
        m["seq"] = np.ascontiguousarray(sc.reshape(2, 128, TP * BC))
        m["mot"] = np.ascontiguousarray(mot[sl].T).astype(bf)
        m["rob"] = np.ascontiguousarray(rob[sl].T).astype(bf)
        m["re_"] = np.ascontiguousarray(real[sl].T).astype(bf)
        m["im_"] = np.ascontiguousarray(imag[sl].T).astype(bf)
        in_maps.append(m)
    return in_maps


def kernel(**inputs):
    global _NC_CACHE
    use_bias_mm = bool(np.any(np.asarray(inputs["bl"])))
    if _NC_CACHE is None or _NC_CACHE[1] != use_bias_mm:
        _NC_CACHE = (build_nc(use_bias_mm), use_bias_mm)
    in_maps = _prep_inputs(inputs)
    res = run_bass_kernel_spmd(_NC_CACHE[0], in_maps, core_ids=list(range(NC)))
    out = np.concatenate(
        [np.asarray(res.results[c]["y"], np.float32).T for c in range(NC)], axis=0)
    return out  # [512, 1] float32


# revision 35
# speedup vs baseline: 1.0526x; 1.0149x over previous
"""Trainium2 Bass kernel for nn_Critic (branch MLPs -> 255-step LSTM -> head).

Strategy (hardcoded, 8 cores, data-parallel over batch B=512 -> 64/core):
  - Feature-major on chip: vectors are [feature_chunk(128), batch(64)].
  - Gate columns of Wk/Wrk/bl are host-permuted to [f, i, 2*g, o] so that
    tanh(g) = 2*sigmoid(2g) - 1 folds into the one sigmoid pass; the
    i*tanh(g) product is then one fused DVE op ((in0*2-1)*in1).
  - PSUM zz[128, parity, subchunk, step, batch]: subchunk order
    [f0,f1,i0,2g0,i1,2g1,o0,o1]; each parity owns 4 physical banks so the
    bank-granular has_written clear of start=True never touches the other
    group's live cells.  zx = Wk^T x is computed 4 steps at a time with
    N=256 matmuls, spread across the previous group's steps.
  - Per-step chain: f-mms -> sigma_f (feeds off-chain c*=sigma_f) ->
    ig-mms -> sigma_ig -> tm1=(2*sig_g-1)*sig_i (fused) -> c+=tm1 ->
    tanh(c) -> h = sigma_o*tanh(c).  sigma_o runs in ACT slack.
  - Keep-warm dummy matmuls gated on tail events stop the PE from
    downclocking during the per-step wait for h.
"""

import os
os.environ.setdefault("TILE_EXHAUSTIVE_MEMORY_SHARE_CHECK", "1")

import numpy as np
import ml_dtypes

import concourse.bass as bass
import concourse.mybir as mybir
import concourse.tile as tile
from concourse import bacc
from concourse.bass_utils import run_bass_kernel_spmd

BF16 = mybir.dt.bfloat16
F32 = mybir.dt.float32
AF = mybir.ActivationFunctionType

NC = 8          # cores
B = 512
BC = B // NC    # 64 batch per core
T = 255         # real steps
TP = 256        # padded steps
NG = TP // 4    # 64 groups of 4 steps
U = 256
DIN = 256

# tuning flags
KW_DUMMIES = os.environ.get("K_KW", "1") == "1"   # keep-warm dummy matmuls
USE_AFM = os.environ.get("K_AFM", "1") == "1"     # fused (2x-1)*y DVE op
WIDE_ZX = os.environ.get("K_WZX", "1") == "1"     # N=256 zx matmuls
SPLIT_SIG = os.environ.get("K_SSIG", "1") == "1"  # sigma_ig + sigma_o split


def build_nc(use_bias_mm=True):
    nc = bacc.Bacc(None, target_bir_lowering=False)

    d_mot = nc.dram_tensor("mot", [64, BC], BF16, kind="ExternalInput")
    d_rob = nc.dram_tensor("rob", [128, BC], BF16, kind="ExternalInput")
    d_re = nc.dram_tensor("re_", [128, BC], BF16, kind="ExternalInput")
    d_im = nc.dram_tensor("im_", [128, BC], BF16, kind="ExternalInput")
    d_seq = nc.dram_tensor("seq", [2, 128, TP * BC], BF16, kind="ExternalInput")
    d_wm = nc.dram_tensor("wm", [64, 256], BF16, kind="ExternalInput")
    d_wr = nc.dram_tensor("wr", [128, 256], BF16, kind="ExternalInput")
    d_wre = nc.dram_tensor("wre", [128, 128], BF16, kind="ExternalInput")
    d_wim = nc.dram_tensor("wim", [128, 128], BF16, kind="ExternalInput")
    d_wc = nc.dram_tensor("wc", [128, 6, 256], BF16, kind="ExternalInput")
    d_wk = nc.dram_tensor("wk", [128, 2, 1024], BF16, kind="ExternalInput")
    d_wrk = nc.dram_tensor("wrk", [128, 2, 1024], BF16, kind="ExternalInput")
    d_wo = nc.dram_tensor("wo", [128, 2, 1], BF16, kind="ExternalInput")
    d_bm = nc.dram_tensor("bm2", [128, 2], F32, kind="ExternalInput")
    d_br = nc.dram_tensor("br2", [128, 2], F32, kind="ExternalInput")
    d_bre = nc.dram_tensor("bre1", [128, 1], F32, kind="ExternalInput")
    d_bim = nc.dram_tensor("bim1", [128, 1], F32, kind="ExternalInput")
    d_bc = nc.dram_tensor("bc2", [128, 2], F32, kind="ExternalInput")
    d_bo = nc.dram_tensor("bo1", [1, 1], F32, kind="ExternalInput")
    d_blw = nc.dram_tensor("blw8", [8, 128], BF16, kind="ExternalInput")
    d_ind = nc.dram_tensor("ind8", [8, 8 * 256], BF16, kind="ExternalInput")
    d_y = nc.dram_tensor("y", [1, BC], F32, kind="ExternalOutput")
    dbg_t = int(os.environ.get("K_DBG_T", "-1"))
    if dbg_t >= 0:
        d_dh = nc.dram_tensor("dh", [128, 2 * BC], BF16, kind="ExternalOutput")
        d_dc = nc.dram_tensor("dc", [128, 2 * BC], F32, kind="ExternalOutput")
        d_dz = nc.dram_tensor("dz", [128, 8, BC], F32, kind="ExternalOutput")
        d_dg = nc.dram_tensor("dg", [128, 8, BC], F32, kind="ExternalOutput")

    with tile.TileContext(nc) as tc:
        with (
            tc.tile_pool(name="sb", bufs=1) as sb,
            tc.tile_pool(name="rot", bufs=3) as rot,
        ):
            t_wk = sb.tile([128, 2, 1024], BF16, tag="wk")
            t_wrk = sb.tile([128, 2, 1024], BF16, tag="wrk")
            t_blw = sb.tile([8, 128], BF16, tag="blw")
            t_ind = sb.tile([8, 8 * 256], BF16, tag="ind")
            t_seq0 = sb.tile([128, TP * BC], BF16, tag="seq0")
            t_seq1 = sb.tile([128, TP * BC], BF16, tag="seq1")
            t_wm = sb.tile([64, 256], BF16, tag="wm")
            t_wr = sb.tile([128, 256], BF16, tag="wr")
            t_wre = sb.tile([128, 128], BF16, tag="wre")
            t_wim = sb.tile([128, 128], BF16, tag="wim")
            t_wc = sb.tile([128, 6, 256], BF16, tag="wc")
            t_wo = sb.tile([128, 2, 1], BF16, tag="wo")
            t_mot = sb.tile([64, BC], BF16, tag="mot")
            t_rob = sb.tile([128, BC], BF16, tag="rob")
            t_re = sb.tile([128, BC], BF16, tag="re")
            t_im = sb.tile([128, BC], BF16, tag="im")
            t_bm = sb.tile([128, 2], F32, tag="bm")
            t_br = sb.tile([128, 2], F32, tag="br")
            t_bre = sb.tile([128, 1], F32, tag="bre")
            t_bim = sb.tile([128, 1], F32, tag="bim")
            t_bc = sb.tile([128, 2], F32, tag="bc")
            t_bo = sb.tile([1, 1], F32, tag="bo")
            t_h = sb.tile([128, 2 * BC], BF16, tag="h")   # h^T, chunk k at cols k*BC
            t_c = sb.tile([128, 2 * BC], F32, tag="c")    # c^T
            t_cat = sb.tile([128, 6, BC], BF16, tag="cat")
            t_y = sb.tile([1, BC], F32, tag="y")

            nc.sync.dma_start(t_seq0[:, 0:8 * BC], d_seq[0, :, 0:8 * BC])
            nc.sync.dma_start(t_seq1[:, 0:8 * BC], d_seq[1, :, 0:8 * BC])
            nc.sync.dma_start(t_wm[:], d_wm[:])
            nc.sync.dma_start(t_wr[:], d_wr[:])
            nc.sync.dma_start(t_wre[:], d_wre[:])
            nc.sync.dma_start(t_wim[:], d_wim[:])
            nc.sync.dma_start(t_wc[:], d_wc[:])
            nc.sync.dma_start(t_mot[:], d_mot[:])
            nc.sync.dma_start(t_rob[:], d_rob[:])
            nc.sync.dma_start(t_re[:], d_re[:])
            nc.sync.dma_start(t_im[:], d_im[:])
            nc.sync.dma_start(t_bm[:], d_bm[:])
            nc.sync.dma_start(t_br[:], d_br[:])
            nc.sync.dma_start(t_bre[:], d_bre[:])
            nc.sync.dma_start(t_bim[:], d_bim[:])
            nc.sync.dma_start(t_bc[:], d_bc[:])
            nc.sync.dma_start(t_bo[:], d_bo[:])
            nc.sync.dma_start(t_wk[:], d_wk[:])
            nc.sync.dma_start(t_wrk[:], d_wrk[:])
            nc.sync.dma_start(t_blw[:], d_blw[:])
            nc.sync.dma_start(t_ind[:], d_ind[:])
            nc.sync.dma_start(t_wo[:], d_wo[:])
            CH = 16 * BC
            nc.sync.dma_start(t_seq0[:, 8 * BC:CH], d_seq[0, :, 8 * BC:CH])
            nc.sync.dma_start(t_seq1[:, 8 * BC:CH], d_seq[1, :, 8 * BC:CH])
            for ch in range(1, TP // 16):
                nc.sync.dma_start(
                    t_seq0[:, ch * CH:(ch + 1) * CH], d_seq[0, :, ch * CH:(ch + 1) * CH])
                nc.sync.dma_start(
                    t_seq1[:, ch * CH:(ch + 1) * CH], d_seq[1, :, ch * CH:(ch + 1) * CH])
            t_seq = [t_seq0, t_seq1]

            # ---- LSTM recurrence ----
            with tc.tile_pool(name="zp", bufs=1, space="PSUM") as zp:
                # zz[part, bank(gate chunk: f0,f1,i0,i1,g0,g1,o0,o1),
                #    group parity, step in group, batch]
                zz = zp.tile([128, 2, 8, 4, BC], F32, tag="zz")

                # front-end branch MLPs use zz parity-1 step-0 cells as PSUM
                # scratch (group 1's region, re-cleared by its zx pack); the
                # group-0 zx prime then overlaps the ACT relu chain.
                for m in range(2):
                    nc.tensor.matmul(zz[:, 1, m, 0, :], t_wm[:, m * 128:(m + 1) * 128],
                                     t_mot[:], start=(m == 0), stop=True,
                                     skip_group_check=True)
                for m in range(2):
                    nc.tensor.matmul(zz[:, 1, 2 + m, 0, :], t_wr[:, m * 128:(m + 1) * 128],
                                     t_rob[:], start=(m == 0), stop=True,
                                     skip_group_check=True)
                nc.tensor.matmul(zz[:, 1, 4, 0, :], t_wre[:], t_re[:], start=True,
                                 stop=True, skip_group_check=True)
                nc.tensor.matmul(zz[:, 1, 5, 0, :], t_wim[:], t_im[:], start=False,
                                 stop=True, skip_group_check=True)
                for m in range(2):
                    nc.scalar.activation(t_cat[:, m, :], zz[:, 1, m, 0, :], AF.Relu,
                                         bias=t_bm[:, m:m + 1])
                for m in range(2):
                    nc.scalar.activation(t_cat[:, 2 + m, :], zz[:, 1, 2 + m, 0, :],
                                         AF.Relu, bias=t_br[:, m:m + 1])
                nc.scalar.activation(t_cat[:, 4, :], zz[:, 1, 4, 0, :], AF.Relu,
                                     bias=t_bre[:, 0:1])
                nc.scalar.activation(t_cat[:, 5, :], zz[:, 1, 5, 0, :], AF.Relu,
                                     bias=t_bim[:, 0:1])

                def emit_zx_pack(gn, banks):
                    """zx (and bias) for group gn, the given bank pairs;
                    N=256 matmuls covering the group's 4 steps at once.
                    Exactly one start=True per physical bank (first write of
                    the even subchunk) -- start clears the whole bank's
                    has_written state, and each parity owns its own banks."""
                    qn = gn % 2
                    for b in banks:
                        if use_bias_mm:
                            for s in (2 * b, 2 * b + 1):
                                nc.tensor.matmul(
                                    zz[:, qn, s], t_blw[:],
                                    t_ind[:, s * 256:(s + 1) * 256],
                                    start=(s == 2 * b), stop=False,
                                    skip_group_check=True)
                        for s in (2 * b, 2 * b + 1):
                            for k in range(2):
                                if WIDE_ZX:
                                    nc.tensor.matmul(
                                        zz[:, qn, s],
                                        t_wk[:, k, s * 128:(s + 1) * 128],
                                        t_seq[k][:, gn * 4 * BC:(gn + 1) * 4 * BC],
                                        start=(not use_bias_mm and k == 0
                                               and s == 2 * b),
                                        stop=False, skip_group_check=True)
                                else:
                                    for p in range(4):
                                        nc.tensor.matmul(
                                            zz[:, qn, s, p, :],
                                            t_wk[:, k, s * 128:(s + 1) * 128],
                                            t_seq[k][:, (gn * 4 + p) * BC:
                                                      (gn * 4 + p + 1) * BC],
                                            start=(not use_bias_mm and k == 0
                                                   and s == 2 * b and p == 0),
                                            stop=False, skip_group_check=True)

                def emit_step(t):
                    g, q, p = t // 4, (t // 4) % 2, t % 4
                    gb = rot.tile([128, 8, BC], BF16, tag="gb")
                    gbo = rot.tile([128, 2, BC], BF16, tag="gbo")
                    tm1 = rot.tile([128, 2, BC], F32, tag="tm1")
                    tm2 = rot.tile([128, 2 * BC], F32, tag="tm2")
                    tmc = rot.tile([128, 2 * BC], BF16, tag="tmc")
                    t_acc = rot.tile([128, 1], F32, tag="acc")
                    if t == dbg_t:
                        dbg_gb.append(gb)
                    # recurrent matmuls: f subchunks first, then i,g, then o
                    for s in range(8):
                        for k in range(2):
                            nc.tensor.matmul(
                                zz[:, q, s, p, :],
                                t_wrk[:, k, s * 128:(s + 1) * 128],
                                t_h[:, k * BC:(k + 1) * BC],
                                start=False, stop=(k == 1),
                                skip_group_check=True)
                    # ACT queue: sigma_f, sigma_igo, tanh(c)
                    nc.scalar.activation(gb[:, 0:2, :], zz[:, q, 0:2, p, :],
                                         AF.Sigmoid)
                    if SPLIT_SIG:
                        nc.scalar.activation(gb[:, 2:6, :], zz[:, q, 2:6, p, :],
                                             AF.Sigmoid)
                        nc.scalar.activation(gbo[:, :, :], zz[:, q, 6:8, p, :],
                                             AF.Sigmoid)
                    else:
                        nc.scalar.activation(gb[:, 2:6, :], zz[:, q, 2:6, p, :],
                                             AF.Sigmoid)
                        nc.scalar.activation(gbo[:, :, :], zz[:, q, 6:8, p, :],
                                             AF.Sigmoid)
                    # DVE: tm2 = sig_f*c ; tm1 = (2*sig_2g-1)*sig_i ;
                    #      c = tm2 + tm1 ; h = sig_o * tanh(c)
                    nc.vector.tensor_mul(tm2[:, :], gb[:, 0:2, :], t_c[:, :])
                    if USE_AFM:
                        nc.vector.affine_mul_reduce(
                            tm1[:, :, :], t_acc[:, 0:1], gb[:, 3:7:2, :],
                            gb[:, 2:6:2, :], 2.0, -1.0)
                    else:
                        nc.vector.tensor_scalar(
                            tm1[:, :, :], gb[:, 3:7:2, :], 2.0, -1.0,
                            mybir.AluOpType.mult, mybir.AluOpType.add)
                        nc.vector.tensor_mul(tm1[:, :, :], tm1[:, :, :],
                                             gb[:, 2:6:2, :])
                    nc.vector.tensor_add(t_c[:, :], tm2[:, :], tm1[:, :, :])
                    nc.scalar.activation(tmc[:, :], t_c[:, :], AF.Tanh)
                    nc.vector.tensor_mul(t_h[:, :], gbo[:, :, :], tmc[:, :])
                    # zx for the next group, one bank pair per step
                    if g + 1 < NG:
                        emit_zx_pack(g + 1, (p,))
                    # keep-warm dummies: tiny matmuls gated on tail results so
                    # the PE never idles long enough to downclock.  They
                    # accumulate (start=False: no bank clear!) onto dead PSUM
                    # cells of this step's already-consumed z.
                    if KW_DUMMIES:
                        # event-gated keep-warm ladder: fat fp32 fillers early
                        # in the tail, short bf16 ones near the h handoff
                        # [event-gated sync, static filler] pairs: the gated
                        # mm aligns to a tail event; static mms behind it in
                        # the in-order queue extend the busy stretch.
                        sf32 = t_seq0[:, 0:128].bitcast(F32)
                        def kw(src_, st_, s, fills):
                            nc.tensor.matmul(
                                zz[0:1, q, s, p, 0:BC], st_, src_,
                                start=False, stop=True, skip_group_check=True)
                            for _ in range(fills):
                                nc.tensor.matmul(
                                    zz[0:1, q, s, p, 0:BC], t_bm[:, 0:1],
                                    sf32[:, 0:BC],
                                    start=False, stop=True,
                                    skip_group_check=True)
                        kw(gb[:, 2, 0:BC], t_wk[:, 0, 0:1], 2, 1)
                        kw(tm1[:, 0, 0:BC], t_bm[:, 0:1], 3, 3)
                        kw(t_c[:, 0:BC], t_bm[:, 0:1], 4, 2)
                        kw(tmc[:, 0:BC], t_wk[:, 0, 0:1], 5, 2)

                dbg_gb = []
                emit_zx_pack(0, range(4))
                # state = relu(cat @ Wc + bc) -> h0, c0 (PE queue: after the
                # zx prime, so the prime overlaps the relu chain above)
                for mo in range(2):
                    for kc in range(6):
                        nc.tensor.matmul(
                            zz[:, 1, 6 + mo, 0, :],
                            t_wc[:, kc, mo * 128:(mo + 1) * 128],
                            t_cat[:, kc, :],
                            start=(mo == 0 and kc == 0), stop=(kc == 5),
                            skip_group_check=True)
                for mo in range(2):
                    nc.scalar.activation(t_h[:, mo * BC:(mo + 1) * BC],
                                         zz[:, 1, 6 + mo, 0, :],
                                         AF.Relu, bias=t_bc[:, mo:mo + 1])
                    nc.scalar.activation(t_c[:, mo * BC:(mo + 1) * BC],
                                         zz[:, 1, 6 + mo, 0, :],
                                         AF.Relu, bias=t_bc[:, mo:mo + 1])
                for t in range(T):
                    emit_step(t)
                    if t == dbg_t:
                        nc.sync.dma_start(d_dh[:], t_h[:])
                        nc.sync.dma_start(d_dc[:], t_c[:])
                        dq, dp = (t // 4) % 2, t % 4
                        dzt = rot.tile([128, 8, BC], F32, tag="dzt")
                        for j in range(8):
                            nc.vector.tensor_copy(dzt[:, j, :], zz[:, dq, j, dp, :])
                        nc.sync.dma_start(d_dz[:], dzt[:])
                        nc.sync.dma_start(d_dg[:], dbg_gb[0][:])

            # ---- output head ----
            with tc.tile_pool(name="hp", bufs=1, space="PSUM") as hp:
                py = hp.tile([1, BC], F32, tag="py")
                for k in range(2):
                    nc.tensor.matmul(py[:], t_wo[:, k, :],
                                     t_h[:, k * BC:(k + 1) * BC],
                                     start=(k == 0), stop=(k == 1))
                nc.scalar.activation(t_y[:], py[:], AF.Relu, bias=t_bo[:, 0:1])
            nc.sync.dma_start(d_y[:], t_y[:])

    nc.compile()
    return nc


_NC_CACHE = None


def _prep_inputs(inputs):
    """Shard + lay out the full-problem inputs into 8 per-core in_maps."""
    bf = ml_dtypes.bfloat16
    f32 = np.float32

    hist = np.asarray(inputs["history"], f32)     # [B, 128, 256]
    act = np.asarray(inputs["action"], f32)       # [B, 128, 256]
    seq = np.concatenate([hist[:, :127], act], axis=1)          # [B, 255, 256]
    seq = np.concatenate(
        [seq, np.zeros((B, 1, DIN), f32)], axis=1)              # [B, 256, 256]

    def gate_perm(w):
        """Permute gate cols [i,f,g,o] -> [f0,f1,i0,2g0,i1,2g1,o0,o1]
        (last axis, 8x128 subchunks)."""
        return np.concatenate(
            [w[..., 256:512], w[..., 0:128], 2.0 * w[..., 512:640],
             w[..., 128:256], 2.0 * w[..., 640:768], w[..., 768:1024]],
            axis=-1)

    Wk = gate_perm(np.asarray(inputs["Wk"], f32))    # [256, 1024]
    Wrk = gate_perm(np.asarray(inputs["Wrk"], f32))
    bl = gate_perm(np.asarray(inputs["bl"], f32).reshape(1, 1024))[0]
    wk_p = np.ascontiguousarray(
        Wk.reshape(2, 128, 1024).transpose(1, 0, 2)).astype(bf)   # [128,2,1024]
    wrk_p = np.ascontiguousarray(
        Wrk.reshape(2, 128, 1024).transpose(1, 0, 2)).astype(bf)
    blw8 = np.ascontiguousarray(bl.reshape(8, 128)).astype(bf)    # [8,128]
    ind8 = np.zeros((8, 8 * 256), f32)
    for j in range(8):
        ind8[j, j * 256:(j + 1) * 256] = 1.0
    ind8 = ind8.astype(bf)
    Wc = np.asarray(inputs["Wc"], f32)            # [768, 256]
    wc_p = np.ascontiguousarray(
        Wc.reshape(6, 128, 256).transpose(1, 0, 2)).astype(bf)    # [128,6,256]
    Wo = np.asarray(inputs["Wo"], f32)            # [256, 1]
    wo_p = np.ascontiguousarray(
        Wo.reshape(2, 128, 1).transpose(1, 0, 2)).astype(bf)      # [128,2,1]

    def bias2(v, chunks):
        return np.ascontiguousarray(np.asarray(v, f32).reshape(chunks, 128).T)

    shared = {
        "wm": np.asarray(inputs["Wm"], f32).astype(bf),
        "wr": np.asarray(inputs["Wr"], f32).astype(bf),
        "wre": np.asarray(inputs["Wre"], f32).astype(bf),
        "wim": np.asarray(inputs["Wim"], f32).astype(bf),
        "wc": wc_p, "wk": wk_p, "wrk": wrk_p, "wo": wo_p,
        "bm2": bias2(inputs["bm"], 2), "br2": bias2(inputs["br"], 2),
        "bre1": bias2(inputs["bre"], 1), "bim1": bias2(inputs["bim"], 1),
        "bc2": bias2(inputs["bc"], 2),
        "bo1": np.asarray(inputs["bo"], f32).reshape(1, 1),
        "blw8": blw8, "ind8": ind8,
    }

    mot = np.asarray(inputs["motion_state"], f32)
    rob = np.asarray(inputs["robot_state"], f32)
    real = np.concatenate([np.asarray(inputs["osc_state_real"], f32),
                           np.asarray(inputs["osc_real"], f32)], -1)
    imag = np.concatenate([np.asarray(inputs["osc_state_imag"], f32),
                           np.asarray(inputs["osc_imag"], f32)], -1)

    in_maps = []
    for c in range(NC):
        sl = slice(c * BC, (c + 1) * BC)
        # on-chip col = t*64 + b  (plain t-major)
        sc = seq[sl].reshape(BC, TP, 2, 128)           # [b, t, fk, fp]
        sc = np.ascontiguousarray(sc.transpose(2, 3, 1, 0)).astype(bf)
        m = dict(shared)
        m["seq"] = np.ascontiguousarray(sc.reshape(2, 128, TP * BC))
        m["mot"] = np.ascontiguousarray(mot[sl].T).astype(bf)
        m["rob"] = np.ascontiguousarray(rob[sl].T).astype(bf)
        m["re_"] = np.ascontiguousarray(real[sl].T).astype(bf)
        m["im_"] = np.ascontiguousarray(imag[sl].T).astype(bf)
        in_maps.append(m)
    return in_maps


def kernel(**inputs):
    global _NC_CACHE
    use_bias_mm = bool(np.any(np.asarray(inputs["bl"])))
    if _NC_CACHE is None or _NC_CACHE[1] != use_bias_mm:
        _NC_CACHE = (build_nc(use_bias_mm), use_bias_mm)
    in_maps = _prep_inputs(inputs)
    res = run_bass_kernel_spmd(_NC_CACHE[0], in_maps, core_ids=list(range(NC)))
    out = np.concatenate(
        [np.asarray(res.results[c]["y"], np.float32).T for c in range(NC)], axis=0)
    return out  # [512, 1] float32
